# revision 31
# baseline (speedup 1.0000x reference)
"""Engram ngram-hash embedding kernel for Trainium2 (8 NeuronCores, Bass/Tile).

Contract: kernel(**inputs) takes the FULL unsharded inputs from
reference.setup_inputs() and returns the FULL [4, 4096, 2048] fp32 output.

Sharding: data-parallel over the 16384 flattened tokens (2048/core); the
~268MB embedding table and the small projections are replicated per core.
Gather row indices (the ngram hash) are precomputed host-side as part of
sharding prep and shipped as an int32 tensor per core.

Per-core device pipeline:
  1. gather: 8 heads x 16 token-tiles indirect-DMA gathers (256B rows),
     issued back-to-back and unthrottled on the gpsimd SWDGE queue (the
     pacing resource: ~1.4us per 128-row gather instruction).
  2. PE transposes emb tiles to f-major; fp16 matmuls (key+value proj,
     fast-weight-load enabled). Value matmuls run before the gate so the
     PE stream never waits on gate math; ungated results stage in SBUF.
  3. RMSNorm-free gate: sim = dot(key, hs) / (sqrt(msK)*sqrt(msQ)*sqrt(H)),
     signed-sqrt + sigmoid; out = gate * staged value, stored fp16.
"""
import math
import os
import numpy as np

import concourse.bass as bass
import concourse.bacc as bacc
import concourse.tile as tile
import concourse.mybir as mybir
from concourse.bass_utils import run_bass_kernel_spmd
from contextlib import ExitStack

P = 128
B, T = 4, 4096
HID = 2048
EH = 512            # engram hidden = 8 heads * 64
PER_HEAD = 64
NHEADS = 8          # total (ngram, head) pairs
NCORES = 8
TPC = (B * T) // NCORES      # tokens per core = 2048
NT = TPC // P                # t-tiles per core = 16
EPS = 1.1920929e-07
AOP = mybir.AluOpType
ACT = mybir.ActivationFunctionType
F32 = mybir.dt.float32
F16 = mybir.dt.float16
I32 = mybir.dt.int32

_cache = {}
last_exec_time_ns = None
last_trace_path = None


def _build(total_rows, use_wkq):
    nc = bacc.Bacc("TRN2", target_bir_lowering=False, debug=False)
    d_hs = nc.dram_tensor("hs", [TPC, HID], F16, kind="ExternalInput").ap()
    d_tab = nc.dram_tensor("tab", [total_rows, PER_HEAD], F16, kind="ExternalInput").ap()
    d_wkv = nc.dram_tensor("wkv", [P, 4 * 2 * HID], F16, kind="ExternalInput").ap()
    d_idx = nc.dram_tensor("idx", [P, P], I32, kind="ExternalInput").ap()
    d_ident = nc.dram_tensor("ident", [P, P], F16, kind="ExternalInput").ap()
    if use_wkq:
        d_wkq = nc.dram_tensor("wkq", [P, HID], F16, kind="ExternalInput").ap()
    d_out = nc.dram_tensor("out", [TPC, HID], F16, kind="ExternalOutput").ap()

    with tile.TileContext(nc) as tc:
        with ExitStack() as ctx:
            cpool = ctx.enter_context(tc.tile_pool(name="cpool", bufs=1))
            embp = ctx.enter_context(tc.tile_pool(name="embp", bufs=NT))
            etp = ctx.enter_context(tc.tile_pool(name="etp", bufs=24))
            hsp = ctx.enter_context(tc.tile_pool(name="hsp", bufs=3))
            vrp = ctx.enter_context(tc.tile_pool(name="vrp", bufs=3))
            outp = ctx.enter_context(tc.tile_pool(name="outp", bufs=NT))
            scrp = ctx.enter_context(tc.tile_pool(name="scrp", bufs=2))
            smp = ctx.enter_context(tc.tile_pool(name="smp", bufs=4))
            pst = ctx.enter_context(tc.tile_pool(name="pst", bufs=2, space="PSUM"))
            psm = ctx.enter_context(tc.tile_pool(name="psm", bufs=6, space="PSUM"))

            # ---------------- prologue ----------------
            idx_t = cpool.tile([P, P], I32)
            nc.sync.dma_start(idx_t[:], d_idx[:])
            ident = cpool.tile([P, P], F16)
            nc.sync.dma_start(ident[:], d_ident[:])

            # weights: fp16, scalar-queue DMA, key-phase chunks first
            wkv = cpool.tile([P, 4 * 2 * HID], F16)
            for ph in range(2):
                for k in range(4):
                    base = k * 2 * HID + ph * HID
                    nc.scalar.dma_start(out=wkv[:, base:base + HID],
                                        in_=d_wkv[:, base:base + HID])

            if use_wkq:
                wkq = cpool.tile([P, HID], F16)
                nc.scalar.dma_start(wkq[:], d_wkq[:])

            # ---------------- gathers: all issued up front, unthrottled ----------------
            emb_tiles = []
            for i in range(NT):
                emb = embp.tile([P, EH], F16, tag="emb")
                for h in range(NHEADS):
                    nc.gpsimd.indirect_dma_start(
                        out=emb[:, h * PER_HEAD:(h + 1) * PER_HEAD],
                        out_offset=None,
                        in_=d_tab[:],
                        in_offset=bass.IndirectOffsetOnAxis(
                            ap=idx_t[:, h * NT + i:h * NT + i + 1], axis=0),
                    )
                emb_tiles.append(emb)

            # ---------------- per-tile: transpose + project + gate ----------------
            inv_hid = 1.0 / HID
            inv_sqrt_hid = 1.0 / math.sqrt(HID)

            GRP = 2
            vo_tiles = []
            for g in range(NT // GRP):
                tiles = range(g * GRP, (g + 1) * GRP)
                vraw_g = {}
                dotg = smp.tile([P, GRP], F32, tag="dotg")
                gsm = smp.tile([P, 2 * GRP], F32, tag="gsm")  # [0:G]=ssqK, [G:2G]=ssqQ
                gateg = smp.tile([P, GRP], F32, tag="gateg")

                # ---- A: transposes + key/value mms + stats (no gate deps) ----
                for i in tiles:
                    j = i - g * GRP
                    emb = emb_tiles[i]
                    hs = hsp.tile([P, HID], F16, tag="hs")
                    nc.sync.dma_start(hs[:], d_hs[i * P:(i + 1) * P, :])
                    if use_wkq:
                        hs_w = hsp.tile([P, HID], F16, tag="hsw")
                        nc.vector.tensor_tensor(hs_w[:], hs[:], wkq[:], op=AOP.mult)
                    else:
                        hs_w = hs

                    embT = []
                    for k in range(4):
                        pstile = pst.tile([P, P], F16, tag="tr", space="PSUM")
                        nc.tensor.transpose(pstile[:], emb[:, k * P:(k + 1) * P], ident[:])
                        et = etp.tile([P, P], F16, tag="et")
                        nc.vector.tensor_copy(et[:], pstile[:])
                        embT.append(et)

                    dotp = smp.tile([P, 4], F32, tag="dotp")
                    mskp = smp.tile([P, 4], F32, tag="mskp")
                    scr = scrp.tile([P, 512], F32, tag="scr")
                    scr2 = scrp.tile([P, 512], F32, tag="scr2")
                    for c in range(4):
                        pm = psm.tile([P, 512], F32, tag="mm", space="PSUM")
                        for k in range(4):
                            nc.tensor.matmul(
                                pm[:], lhsT=embT[k][:],
                                rhs=wkv[:, k * 2 * HID + c * 512:k * 2 * HID + (c + 1) * 512],
                                start=(k == 0), stop=(k == 3))
                        nc.vector.scalar_tensor_tensor(
                            out=scr[:], in0=pm[:], scalar=1.0,
                            in1=hs_w[:, c * 512:(c + 1) * 512],
                            op0=AOP.mult, op1=AOP.mult, accum_out=dotp[:, c:c + 1])
                        nc.scalar.activation(scr2[:], pm[:], ACT.Square,
                                             accum_out=mskp[:, c:c + 1])
                    nc.vector.tensor_tensor(dotp[:, 0:1], dotp[:, 0:1], dotp[:, 1:2], op=AOP.add)
                    nc.vector.tensor_tensor(dotp[:, 2:3], dotp[:, 2:3], dotp[:, 3:4], op=AOP.add)
                    nc.vector.tensor_tensor(dotg[:, j:j + 1], dotp[:, 0:1], dotp[:, 2:3], op=AOP.add)
                    nc.vector.tensor_tensor(mskp[:, 0:1], mskp[:, 0:1], mskp[:, 1:2], op=AOP.add)
                    nc.vector.tensor_tensor(mskp[:, 2:3], mskp[:, 2:3], mskp[:, 3:4], op=AOP.add)
                    nc.vector.tensor_tensor(gsm[:, j:j + 1], mskp[:, 0:1], mskp[:, 2:3], op=AOP.add)

                    # msQ (feeds only the gate; emitted late on the DVE so it
                    # never sits ahead of PE-critical DVE work in the queue)
                    hsq_scr = scrp.tile([P, HID], F32, tag="hsq", bufs=1)
                    nc.vector.scalar_tensor_tensor(
                        out=hsq_scr[:], in0=hs[:], scalar=1.0, in1=hs[:],
                        op0=AOP.mult, op1=AOP.mult, accum_out=gsm[:, GRP + j:GRP + j + 1])

                    # value mms now (ungated), staged to SBUF; pm drain split
                    # across scalar+DVE so neither engine gates PSUM reuse
                    vraw = vrp.tile([P, HID], F32, tag="vraw")
                    for c in range(4):
                        pm = psm.tile([P, 512], F32, tag="mm", space="PSUM")
                        for k in range(4):
                            nc.tensor.matmul(
                                pm[:], lhsT=embT[k][:],
                                rhs=wkv[:, k * 2 * HID + HID + c * 512:
                                        k * 2 * HID + HID + (c + 1) * 512],
                                start=(k == 0), stop=(k == 3))
                        if c < 2:
                            nc.scalar.activation(vraw[:, c * 512:(c + 1) * 512], pm[:],
                                                 ACT.Copy)
                        else:
                            nc.vector.tensor_copy(vraw[:, c * 512:(c + 1) * 512], pm[:])
                    vraw_g[i] = vraw

                # ---- B: batched gate math on [128, GRP] ----
                # rsqrt/sqrt via magic-constant + 2 Newton steps on the DVE
                # (table-free; keeps Sigmoid as the only scalar activation so
                # its table stays resident instead of thrashing per group).
                def rsqrt_dve(y, x, tmps):
                    t1, t, t2, u = tmps
                    nc.vector.tensor_scalar(t1[:].bitcast(I32), x[:].bitcast(I32),
                                            1, None, op0=AOP.logical_shift_right)
                    nc.vector.tensor_scalar(y[:].bitcast(I32), t1[:].bitcast(I32),
                                            -1, 0x5f3759df, op0=AOP.mult, op1=AOP.add)
                    # one Newton step: ~1.7e-3 rel, ample for the gate
                    nc.vector.tensor_tensor(t[:], x[:], y[:], op=AOP.mult)
                    nc.vector.tensor_tensor(t2[:], t[:], y[:], op=AOP.mult)
                    nc.vector.tensor_scalar(u[:], t2[:], -0.5, 1.5,
                                            op0=AOP.mult, op1=AOP.add)
                    nc.vector.tensor_tensor(y[:], y[:], u[:], op=AOP.mult)

                tmps = [smp.tile([P, GRP], F32, name=f"rt{n}", tag=f"rt{n}")
                        for n in range(4)]
                nc.vector.tensor_scalar(gsm[:], gsm[:], inv_hid, EPS,
                                        op0=AOP.mult, op1=AOP.add)
                den = smp.tile([P, GRP], F32, tag="den")
                nc.vector.tensor_tensor(den[:], gsm[:, 0:GRP], gsm[:, GRP:2 * GRP], op=AOP.mult)
                rden = smp.tile([P, GRP], F32, tag="rden")
                rsqrt_dve(rden, den, tmps)
                sim = smp.tile([P, GRP], F32, tag="sim")
                nc.vector.scalar_tensor_tensor(
                    out=sim[:], in0=dotg[:], scalar=inv_sqrt_hid, in1=rden[:],
                    op0=AOP.mult, op1=AOP.mult)
                av = smp.tile([P, GRP], F32, tag="av")
                nc.vector.tensor_scalar(av[:].bitcast(I32), sim[:].bitcast(I32),
                                        0x7FFFFFFF, None, op0=AOP.bitwise_and)
                nc.vector.tensor_scalar(av[:], av[:], 1e-6, None, op0=AOP.max)
                rav = smp.tile([P, GRP], F32, tag="rav")
                rsqrt_dve(rav, av, tmps)
                nc.vector.tensor_tensor(av[:], av[:], rav[:], op=AOP.mult)
                sgn = smp.tile([P, GRP], F32, tag="sgn")
                nc.vector.tensor_scalar(sgn[:].bitcast(I32), sim[:].bitcast(I32),
                                        -0x80000000, None, op0=AOP.bitwise_and)
                nc.vector.tensor_tensor(gateg[:].bitcast(I32), av[:].bitcast(I32),
                                        sgn[:].bitcast(I32), op=AOP.bitwise_or)
                nc.scalar.activation(gateg[:], gateg[:], ACT.Sigmoid)

                # ---- C: gated scale of staged values (stores deferred) ----
                for i in tiles:
                    j = i - g * GRP
                    vraw = vraw_g[i]
                    vo = outp.tile([P, HID], F16, tag="vo")
                    for c in range(4):
                        if c < 2:
                            nc.scalar.activation(vo[:, c * 512:(c + 1) * 512],
                                                 vraw[:, c * 512:(c + 1) * 512],
                                                 ACT.Copy, scale=gateg[:, j:j + 1])
                        else:
                            nc.vector.tensor_scalar(vo[:, c * 512:(c + 1) * 512],
                                                    vraw[:, c * 512:(c + 1) * 512],
                                                    gateg[:, j:j + 1], None, op0=AOP.mult)
                    vo_tiles.append((i, vo))

            # ---- out-stores on the gpsimd queue, queued BEHIND the gathers:
            # the in-order SWDGE queue keeps store DMA traffic out of the
            # gather window (stores polluting it cost ~20us of ring stalls),
            # and Q7 is idle once the gathers are done.
            for i, vo in vo_tiles:
                nc.gpsimd.dma_start(d_out[i * P:(i + 1) * P, :], vo[:])
    nc.compile()
    return nc


def _hash_idx(input_ids, mults, mods, offsets):
    """Host-side ngram hash (int64 wraparound, exactly the reference math).

    Returns flat row indices [B*T, NHEADS] int32 (head-table offsets applied).
    """
    ids = np.asarray(input_ids).astype(np.int64)
    mults = np.asarray(mults).astype(np.int64)
    mods = np.asarray(mods).astype(np.int64)
    offsets = np.asarray(offsets).astype(np.int64)
    t = ids.shape[1]
    sh1 = np.zeros_like(ids)
    sh1[:, 1:] = ids[:, :-1]
    sh2 = np.zeros_like(ids)
    sh2[:, 2:] = ids[:, :-2]
    with np.errstate(over="ignore"):
        mix2 = (ids * mults[0]) ^ (sh1 * mults[1])
        mix3 = mix2 ^ (sh2 * mults[2])
    idx = np.empty((ids.shape[0], t, NHEADS), np.int64)
    for h in range(NHEADS):
        mix = mix2 if h < 4 else mix3
        idx[:, :, h] = np.remainder(mix, mods[h]) + offsets[h]
    return idx.reshape(-1, NHEADS).astype(np.int32)


def _prep(hidden_states, input_ids, emb_table, Wk, Wv, key_norm_w, query_norm_w,
          offsets, mults, mods):
    """Host-side layout/sharding prep. Returns (in_maps, total_rows, use_wkq)."""
    idx_flat = _hash_idx(input_ids, mults, mods, offsets)  # [B*T, 8]

    # weights [128, 16384] fp16: wkv[p, k*4096 + phase*2048 + d] = W{k/v}[d, 128k+p]
    Wk = np.asarray(Wk, np.float32)
    Wv = np.asarray(Wv, np.float32)
    wkv = np.zeros((P, 4 * 2 * HID), np.float16)
    for k in range(4):
        wkv[:, k * 4096:k * 4096 + HID] = Wk[:, P * k:P * (k + 1)].T.astype(np.float16)
        wkv[:, k * 4096 + HID:(k + 1) * 4096] = Wv[:, P * k:P * (k + 1)].T.astype(np.float16)

    wkq = (np.asarray(key_norm_w, np.float32) * np.asarray(query_norm_w, np.float32))
    use_wkq = not np.allclose(wkq, 1.0)
    wkq_b = np.broadcast_to(wkq.astype(np.float16), (P, HID)).copy() if use_wkq else None

    tab = np.ascontiguousarray(np.asarray(emb_table, np.float32).astype(np.float16))
    total_rows = tab.shape[0]
    hs_flat = np.ascontiguousarray(
        np.asarray(hidden_states, np.float32).reshape(B * T, HID).astype(np.float16))
    ident = np.eye(P, dtype=np.float16)

    in_maps = []
    for c in range(NCORES):
        # idx tile [128, 128]: idx[p, h*NT + i] = row for token c*TPC + i*128 + p
        blk = idx_flat[c * TPC:(c + 1) * TPC].reshape(NT, P, NHEADS)
        idx_tile = np.ascontiguousarray(
            blk.transpose(1, 2, 0).reshape(P, NHEADS * NT)).astype(np.int32)
        m = {
            "hs": np.ascontiguousarray(hs_flat[c * TPC:(c + 1) * TPC]),
            "tab": tab,
            "wkv": wkv,
            "idx": idx_tile,
            "ident": ident,
        }
        if use_wkq:
            m["wkq"] = wkq_b
        in_maps.append(m)
    return in_maps, total_rows, use_wkq


def kernel(hidden_states, input_ids, emb_table, Wk, Wv, key_norm_w, query_norm_w,
           offsets, mults, mods):
    global last_exec_time_ns, last_trace_path
    in_maps, total_rows, use_wkq = _prep(
        hidden_states, input_ids, emb_table, Wk, Wv, key_norm_w, query_norm_w,
        offsets, mults, mods)

    key = (total_rows, use_wkq)
    if key not in _cache:
        _cache[key] = _build(total_rows, use_wkq)
    nc = _cache[key]

    trace = bool(int(os.environ.get("ENGRAM_TRACE", "0")))
    if trace:
        try:
            import ntff_hook  # noqa: F401  (dev-only profiling helper)
        except ImportError:
            trace = False
    res = run_bass_kernel_spmd(nc, in_maps, core_ids=list(range(NCORES)), trace=trace)
    last_exec_time_ns = res.exec_time_ns
    if res.instructions_and_trace:
        last_trace_path = res.instructions_and_trace[1]

    out = np.concatenate([res.results[c]["out"] for c in range(NCORES)], axis=0)
    return out.reshape(B, T, HID).astype(np.float32)


# revision 32
# speedup vs baseline: 1.1797x; 1.1797x over previous
"""Engram ngram-hash embedding kernel for Trainium2 (8 NeuronCores, Bass/Tile).

Contract: kernel(**inputs) takes the FULL unsharded inputs from
reference.setup_inputs() and returns the FULL [4, 4096, 2048] fp32 output.

Sharding: data-parallel over the 16384 flattened tokens (2048/core); the
~268MB embedding table and the small projections are replicated per core.
Gather row indices (the ngram hash) are precomputed host-side as part of
sharding prep and shipped as an int32 tensor per core.

Per-core device pipeline:
  1. gather: 8 heads x 16 token-tiles indirect-DMA gathers (256B rows),
     issued back-to-back and unthrottled on the gpsimd SWDGE queue (the
     pacing resource: ~1.4us per 128-row gather instruction).
  2. PE transposes emb tiles to f-major; fp16 matmuls (key+value proj,
     fast-weight-load enabled). Value matmuls run before the gate so the
     PE stream never waits on gate math; ungated results stage in SBUF.
  3. RMSNorm-free gate: sim = dot(key, hs) / (sqrt(msK)*sqrt(msQ)*sqrt(H)),
     signed-sqrt + sigmoid; out = gate * staged value, stored fp16.
"""
import math
import os
import numpy as np

import concourse.bass as bass
import concourse.bacc as bacc
import concourse.tile as tile
import concourse.mybir as mybir
from concourse.bass_utils import run_bass_kernel_spmd
from contextlib import ExitStack

P = 128
B, T = 4, 4096
HID = 2048
EH = 512            # engram hidden = 8 heads * 64
PER_HEAD = 64
NHEADS = 8          # total (ngram, head) pairs
NCORES = 8
TPC = (B * T) // NCORES      # tokens per core = 2048
NT = TPC // P                # t-tiles per core = 16
EPS = 1.1920929e-07
AOP = mybir.AluOpType
ACT = mybir.ActivationFunctionType
F32 = mybir.dt.float32
F16 = mybir.dt.float16
I32 = mybir.dt.int32

_cache = {}
last_exec_time_ns = None
last_trace_path = None


def _build(total_rows, use_wkq):
    nc = bacc.Bacc("TRN2", target_bir_lowering=False, debug=False)
    d_hs = nc.dram_tensor("hs", [TPC, HID], F16, kind="ExternalInput").ap()
    d_tab = nc.dram_tensor("tab", [total_rows, PER_HEAD], F16, kind="ExternalInput").ap()
    d_wkv = nc.dram_tensor("wkv", [P, 4 * 2 * HID], F16, kind="ExternalInput").ap()
    d_idx = nc.dram_tensor("idx", [P, P], I32, kind="ExternalInput").ap()
    d_ident = nc.dram_tensor("ident", [P, P], F16, kind="ExternalInput").ap()
    if use_wkq:
        d_wkq = nc.dram_tensor("wkq", [P, HID], F16, kind="ExternalInput").ap()
    d_out = nc.dram_tensor("out", [TPC, HID], F16, kind="ExternalOutput").ap()

    with tile.TileContext(nc) as tc:
        with ExitStack() as ctx:
            cpool = ctx.enter_context(tc.tile_pool(name="cpool", bufs=1))
            embp = ctx.enter_context(tc.tile_pool(name="embp", bufs=NT))
            etp = ctx.enter_context(tc.tile_pool(name="etp", bufs=24))
            hsp = ctx.enter_context(tc.tile_pool(name="hsp", bufs=3))
            vrp = ctx.enter_context(tc.tile_pool(name="vrp", bufs=3))
            outp = ctx.enter_context(tc.tile_pool(name="outp", bufs=3))
            scrp = ctx.enter_context(tc.tile_pool(name="scrp", bufs=2))
            smp = ctx.enter_context(tc.tile_pool(name="smp", bufs=4))
            pst = ctx.enter_context(tc.tile_pool(name="pst", bufs=2, space="PSUM"))
            psm = ctx.enter_context(tc.tile_pool(name="psm", bufs=6, space="PSUM"))

            # ---------------- prologue ----------------
            idx_t = cpool.tile([P, P], I32)
            nc.sync.dma_start(idx_t[:], d_idx[:])
            ident = cpool.tile([P, P], F16)
            nc.sync.dma_start(ident[:], d_ident[:])

            # weights: fp16, scalar-queue DMA, key-phase chunks first
            wkv = cpool.tile([P, 4 * 2 * HID], F16)
            for ph in range(2):
                for k in range(4):
                    base = k * 2 * HID + ph * HID
                    nc.scalar.dma_start(out=wkv[:, base:base + HID],
                                        in_=d_wkv[:, base:base + HID])

            if use_wkq:
                wkq = cpool.tile([P, HID], F16)
                nc.scalar.dma_start(wkq[:], d_wkq[:])

            # ---------------- gathers: all issued up front, unthrottled ----------------
            emb_tiles = []
            for i in range(NT):
                emb = embp.tile([P, EH], F16, tag="emb")
                for h in range(NHEADS):
                    nc.gpsimd.indirect_dma_start(
                        out=emb[:, h * PER_HEAD:(h + 1) * PER_HEAD],
                        out_offset=None,
                        in_=d_tab[:],
                        in_offset=bass.IndirectOffsetOnAxis(
                            ap=idx_t[:, h * NT + i:h * NT + i + 1], axis=0),
                    )
                emb_tiles.append(emb)

            # ---------------- per-tile: transpose + project + gate ----------------
            inv_hid = 1.0 / HID
            inv_sqrt_hid = 1.0 / math.sqrt(HID)

            GRP = 2
            for g in range(NT // GRP):
                tiles = range(g * GRP, (g + 1) * GRP)
                vraw_g = {}
                dotg = smp.tile([P, GRP], F32, tag="dotg")
                gsm = smp.tile([P, 2 * GRP], F32, tag="gsm")  # [0:G]=ssqK, [G:2G]=ssqQ
                gateg = smp.tile([P, GRP], F32, tag="gateg")

                # ---- A: transposes + key/value mms + stats (no gate deps) ----
                for i in tiles:
                    j = i - g * GRP
                    emb = emb_tiles[i]
                    hs = hsp.tile([P, HID], F16, tag="hs")
                    nc.sync.dma_start(hs[:], d_hs[i * P:(i + 1) * P, :])
                    if use_wkq:
                        hs_w = hsp.tile([P, HID], F16, tag="hsw")
                        nc.vector.tensor_tensor(hs_w[:], hs[:], wkq[:], op=AOP.mult)
                    else:
                        hs_w = hs

                    embT = []
                    for k in range(4):
                        pstile = pst.tile([P, P], F16, tag="tr", space="PSUM")
                        nc.tensor.transpose(pstile[:], emb[:, k * P:(k + 1) * P], ident[:])
                        et = etp.tile([P, P], F16, tag="et")
                        nc.vector.tensor_copy(et[:], pstile[:])
                        embT.append(et)

                    dotp = smp.tile([P, 4], F32, tag="dotp")
                    mskp = smp.tile([P, 4], F32, tag="mskp")
                    scr = scrp.tile([P, 512], F32, tag="scr")
                    scr2 = scrp.tile([P, 512], F32, tag="scr2")
                    for c in range(4):
                        pm = psm.tile([P, 512], F32, tag="mm", space="PSUM")
                        for k in range(4):
                            nc.tensor.matmul(
                                pm[:], lhsT=embT[k][:],
                                rhs=wkv[:, k * 2 * HID + c * 512:k * 2 * HID + (c + 1) * 512],
                                start=(k == 0), stop=(k == 3))
                        nc.vector.scalar_tensor_tensor(
                            out=scr[:], in0=pm[:], scalar=1.0,
                            in1=hs_w[:, c * 512:(c + 1) * 512],
                            op0=AOP.mult, op1=AOP.mult, accum_out=dotp[:, c:c + 1])
                        nc.scalar.activation(scr2[:], pm[:], ACT.Square,
                                             accum_out=mskp[:, c:c + 1])
                    nc.vector.tensor_tensor(dotp[:, 0:1], dotp[:, 0:1], dotp[:, 1:2], op=AOP.add)
                    nc.vector.tensor_tensor(dotp[:, 2:3], dotp[:, 2:3], dotp[:, 3:4], op=AOP.add)
                    nc.vector.tensor_tensor(dotg[:, j:j + 1], dotp[:, 0:1], dotp[:, 2:3], op=AOP.add)
                    nc.vector.tensor_tensor(mskp[:, 0:1], mskp[:, 0:1], mskp[:, 1:2], op=AOP.add)
                    nc.vector.tensor_tensor(mskp[:, 2:3], mskp[:, 2:3], mskp[:, 3:4], op=AOP.add)
                    nc.vector.tensor_tensor(gsm[:, j:j + 1], mskp[:, 0:1], mskp[:, 2:3], op=AOP.add)

                    # msQ (feeds only the gate; emitted late on the DVE so it
                    # never sits ahead of PE-critical DVE work in the queue)
                    hsq_scr = scrp.tile([P, HID], F32, tag="hsq", bufs=1)
                    nc.vector.scalar_tensor_tensor(
                        out=hsq_scr[:], in0=hs[:], scalar=1.0, in1=hs[:],
                        op0=AOP.mult, op1=AOP.mult, accum_out=gsm[:, GRP + j:GRP + j + 1])

                    # value mms now (ungated), staged to SBUF; pm drain split
                    # across scalar+DVE so neither engine gates PSUM reuse
                    vraw = vrp.tile([P, HID], F32, tag="vraw")
                    for c in range(4):
                        pm = psm.tile([P, 512], F32, tag="mm", space="PSUM")
                        for k in range(4):
                            nc.tensor.matmul(
                                pm[:], lhsT=embT[k][:],
                                rhs=wkv[:, k * 2 * HID + HID + c * 512:
                                        k * 2 * HID + HID + (c + 1) * 512],
                                start=(k == 0), stop=(k == 3))
                        if c < 2:
                            nc.scalar.activation(vraw[:, c * 512:(c + 1) * 512], pm[:],
                                                 ACT.Copy)
                        else:
                            nc.vector.tensor_copy(vraw[:, c * 512:(c + 1) * 512], pm[:])
                    vraw_g[i] = vraw

                # ---- B: batched gate math on [128, GRP] ----
                # rsqrt/sqrt via magic-constant + 2 Newton steps on the DVE
                # (table-free; keeps Sigmoid as the only scalar activation so
                # its table stays resident instead of thrashing per group).
                def rsqrt_dve(y, x, tmps):
                    t1, t, t2, u = tmps
                    nc.vector.tensor_scalar(t1[:].bitcast(I32), x[:].bitcast(I32),
                                            1, None, op0=AOP.logical_shift_right)
                    nc.vector.tensor_scalar(y[:].bitcast(I32), t1[:].bitcast(I32),
                                            -1, 0x5f3759df, op0=AOP.mult, op1=AOP.add)
                    # one Newton step: ~1.7e-3 rel, ample for the gate
                    nc.vector.tensor_tensor(t[:], x[:], y[:], op=AOP.mult)
                    nc.vector.tensor_tensor(t2[:], t[:], y[:], op=AOP.mult)
                    nc.vector.tensor_scalar(u[:], t2[:], -0.5, 1.5,
                                            op0=AOP.mult, op1=AOP.add)
                    nc.vector.tensor_tensor(y[:], y[:], u[:], op=AOP.mult)

                tmps = [smp.tile([P, GRP], F32, name=f"rt{n}", tag=f"rt{n}")
                        for n in range(4)]
                nc.vector.tensor_scalar(gsm[:], gsm[:], inv_hid, EPS,
                                        op0=AOP.mult, op1=AOP.add)
                den = smp.tile([P, GRP], F32, tag="den")
                nc.vector.tensor_tensor(den[:], gsm[:, 0:GRP], gsm[:, GRP:2 * GRP], op=AOP.mult)
                rden = smp.tile([P, GRP], F32, tag="rden")
                rsqrt_dve(rden, den, tmps)
                sim = smp.tile([P, GRP], F32, tag="sim")
                nc.vector.scalar_tensor_tensor(
                    out=sim[:], in0=dotg[:], scalar=inv_sqrt_hid, in1=rden[:],
                    op0=AOP.mult, op1=AOP.mult)
                av = smp.tile([P, GRP], F32, tag="av")
                nc.vector.tensor_scalar(av[:].bitcast(I32), sim[:].bitcast(I32),
                                        0x7FFFFFFF, None, op0=AOP.bitwise_and)
                nc.vector.tensor_scalar(av[:], av[:], 1e-6, None, op0=AOP.max)
                rav = smp.tile([P, GRP], F32, tag="rav")
                rsqrt_dve(rav, av, tmps)
                nc.vector.tensor_tensor(av[:], av[:], rav[:], op=AOP.mult)
                sgn = smp.tile([P, GRP], F32, tag="sgn")
                nc.vector.tensor_scalar(sgn[:].bitcast(I32), sim[:].bitcast(I32),
                                        -0x80000000, None, op0=AOP.bitwise_and)
                nc.vector.tensor_tensor(gateg[:].bitcast(I32), av[:].bitcast(I32),
                                        sgn[:].bitcast(I32), op=AOP.bitwise_or)
                nc.scalar.activation(gateg[:], gateg[:], ACT.Sigmoid)

                # ---- C: gated scale of staged values + out (quarter DMAs) ----
                for i in tiles:
                    j = i - g * GRP
                    vraw = vraw_g[i]
                    vo = outp.tile([P, HID], F16, tag="vo")
                    for c in range(4):
                        if c < 2:
                            nc.scalar.activation(vo[:, c * 512:(c + 1) * 512],
                                                 vraw[:, c * 512:(c + 1) * 512],
                                                 ACT.Copy, scale=gateg[:, j:j + 1])
                        else:
                            nc.vector.tensor_scalar(vo[:, c * 512:(c + 1) * 512],
                                                    vraw[:, c * 512:(c + 1) * 512],
                                                    gateg[:, j:j + 1], None, op0=AOP.mult)
                        nc.sync.dma_start(d_out[i * P:(i + 1) * P, c * 512:(c + 1) * 512],
                                          vo[:, c * 512:(c + 1) * 512])
    nc.compile()
    return nc


def _hash_idx(input_ids, mults, mods, offsets):
    """Host-side ngram hash (int64 wraparound, exactly the reference math).

    Returns flat row indices [B*T, NHEADS] int32 (head-table offsets applied).
    """
    ids = np.asarray(input_ids).astype(np.int64)
    mults = np.asarray(mults).astype(np.int64)
    mods = np.asarray(mods).astype(np.int64)
    offsets = np.asarray(offsets).astype(np.int64)
    t = ids.shape[1]
    sh1 = np.zeros_like(ids)
    sh1[:, 1:] = ids[:, :-1]
    sh2 = np.zeros_like(ids)
    sh2[:, 2:] = ids[:, :-2]
    with np.errstate(over="ignore"):
        mix2 = (ids * mults[0]) ^ (sh1 * mults[1])
        mix3 = mix2 ^ (sh2 * mults[2])
    idx = np.empty((ids.shape[0], t, NHEADS), np.int64)
    for h in range(NHEADS):
        mix = mix2 if h < 4 else mix3
        idx[:, :, h] = np.remainder(mix, mods[h]) + offsets[h]
    return idx.reshape(-1, NHEADS).astype(np.int32)


def _prep(hidden_states, input_ids, emb_table, Wk, Wv, key_norm_w, query_norm_w,
          offsets, mults, mods):
    """Host-side layout/sharding prep. Returns (in_maps, total_rows, use_wkq)."""
    idx_flat = _hash_idx(input_ids, mults, mods, offsets)  # [B*T, 8]

    # weights [128, 16384] fp16: wkv[p, k*4096 + phase*2048 + d] = W{k/v}[d, 128k+p]
    Wk = np.asarray(Wk, np.float32)
    Wv = np.asarray(Wv, np.float32)
    wkv = np.zeros((P, 4 * 2 * HID), np.float16)
    for k in range(4):
        wkv[:, k * 4096:k * 4096 + HID] = Wk[:, P * k:P * (k + 1)].T.astype(np.float16)
        wkv[:, k * 4096 + HID:(k + 1) * 4096] = Wv[:, P * k:P * (k + 1)].T.astype(np.float16)

    wkq = (np.asarray(key_norm_w, np.float32) * np.asarray(query_norm_w, np.float32))
    use_wkq = not np.allclose(wkq, 1.0)
    wkq_b = np.broadcast_to(wkq.astype(np.float16), (P, HID)).copy() if use_wkq else None

    tab = np.ascontiguousarray(np.asarray(emb_table, np.float32).astype(np.float16))
    total_rows = tab.shape[0]
    hs_flat = np.ascontiguousarray(
        np.asarray(hidden_states, np.float32).reshape(B * T, HID).astype(np.float16))
    ident = np.eye(P, dtype=np.float16)

    in_maps = []
    for c in range(NCORES):
        # idx tile [128, 128]: idx[p, h*NT + i] = row for token c*TPC + i*128 + p
        blk = idx_flat[c * TPC:(c + 1) * TPC].reshape(NT, P, NHEADS)
        idx_tile = np.ascontiguousarray(
            blk.transpose(1, 2, 0).reshape(P, NHEADS * NT)).astype(np.int32)
        m = {
            "hs": np.ascontiguousarray(hs_flat[c * TPC:(c + 1) * TPC]),
            "tab": tab,
            "wkv": wkv,
            "idx": idx_tile,
            "ident": ident,
        }
        if use_wkq:
            m["wkq"] = wkq_b
        in_maps.append(m)
    return in_maps, total_rows, use_wkq


def kernel(hidden_states, input_ids, emb_table, Wk, Wv, key_norm_w, query_norm_w,
           offsets, mults, mods):
    global last_exec_time_ns, last_trace_path
    in_maps, total_rows, use_wkq = _prep(
        hidden_states, input_ids, emb_table, Wk, Wv, key_norm_w, query_norm_w,
        offsets, mults, mods)

    key = (total_rows, use_wkq)
    if key not in _cache:
        _cache[key] = _build(total_rows, use_wkq)
    nc = _cache[key]

    trace = bool(int(os.environ.get("ENGRAM_TRACE", "0")))
    if trace:
        try:
            import ntff_hook  # noqa: F401  (dev-only profiling helper)
        except ImportError:
            trace = False
    res = run_bass_kernel_spmd(nc, in_maps, core_ids=list(range(NCORES)), trace=trace)
    last_exec_time_ns = res.exec_time_ns
    if res.instructions_and_trace:
        last_trace_path = res.instructions_and_trace[1]

    out = np.concatenate([res.results[c]["out"] for c in range(NCORES)], axis=0)
    return out.reshape(B, T, HID).astype(np.float32)
